# revision 1
# baseline (speedup 1.0000x reference)
"""TRN2 Bass kernel for nn_MultiHeadSeqAttention (B=8, M=1024, H=1024, 16 heads).

Reference computes out = ((h Wq^T) (h Wk^T)^T) (h Wv^T) per head, then Wo^T.
There is NO softmax, so the attention product reassociates:
    out_h = q_h @ (k_h^T @ v_h)        with  k_h^T v_h : [64, 64]
which removes the [M, M] score matrix entirely. Per core (1 batch):
4 dense 1024^3 GEMMs + 16 tiny per-head [64x64] products, ~8.9 GFLOP.

Sharding: data-parallel over B across 8 cores; no collectives.
Host pre-transposes h and the weights so every matmul contraction is on the
partition dim (no on-chip transposes).

Precision: the q/k/v path runs in bf16 (q, k, v are consumed in bf16 by the
S / o stages anyway; halves the startup DMA) and so does the final
projection (bf16 LDWEIGHTS uses fast-weight-load: 216 vs 227 ns/matmul).
fp32 accumulation everywhere. Measured end-to-end absmax rel err ~4.3e-3.

Per-core layouts (i = input feature, o = q/k/v feature, j = out feature):
    ht  [i, m] = h[b].T          wq/wk/wv [i, o] = W.T        wo [o, j] = Wo.T
    k = ht.T @ wk : [m, o]       v : [m, o]      (partition = m)
    S_h = k_h^T @ v_h : [64,64]  all 16 heads packed in one [128,512] PSUM bank
    qT = wq.T @ ht : [o, m]      oT_h = S_h^T @ qT_h : [o, m] (partition = o)
    out = oT.T @ wo : [m, j]     (partition = m, DMA'd straight out)
"""

import numpy as np
import ml_dtypes

import concourse.bass as bass
import concourse.mybir as mybir
import concourse.tile as tile
from concourse import bacc
from concourse.bass_utils import run_bass_kernel_spmd

F32R = mybir.dt.float32r
F32 = mybir.dt.float32
BF16 = mybir.dt.bfloat16

P = 128          # partitions
H = 1024         # model dim
M = 1024         # sequence length
NT = H // P      # 8 tiles of 128 along any 1024 dim
D = 64           # head dim
NH = 16          # heads
NC = 8           # cores
FD = 512         # matmul moving free dim
WARMUP_MM = 46   # PE warmup matmuls (trip the HAM clock gate during DMA-in)

_CACHE = {}


def _build():
    nc = bacc.Bacc("TRN2", target_bir_lowering=False, debug=False,
                   num_devices=NC, enable_asserts=False)

    ht_d = nc.dram_tensor("ht", [H, M], BF16, kind="ExternalInput")
    wq_d = nc.dram_tensor("wq", [H, H], BF16, kind="ExternalInput")
    wk_d = nc.dram_tensor("wk", [H, H], BF16, kind="ExternalInput")
    wv_d = nc.dram_tensor("wv", [H, H], BF16, kind="ExternalInput")
    wo_d = nc.dram_tensor("wo", [H, H], BF16, kind="ExternalInput")
    out_d = nc.dram_tensor("out", [M, H], F32, kind="ExternalOutput")

    with tile.TileContext(nc) as tc:
        with tc.tile_pool(name="sb", bufs=1) as sb, \
             tc.tile_pool(name="ps", bufs=1, space="PSUM") as ps:

            # ---- PE warmup: dep-free matmuls issued while DMAs stream in,
            # so the HAM clock gate reaches 8/8 before real work arrives.
            # Operands are the framework's preamble-memset const tensors
            # (broadcast APs), so the first matmul has zero dependencies ----
            wu_lhs = nc.const_aps.tensor(1.0, [P, P], BF16)
            wu_rhs = nc.const_aps.tensor(1.0, [P, FD], BF16)
            wu_ps = ps.tile([P, FD], F32, tag="big", bufs=7, name="wu_ps")
            for _ in range(WARMUP_MM):
                nc.tensor.matmul(wu_ps[:], wu_lhs, wu_rhs,
                                 start=True, stop=True)

            # ---- loads, in need order: (ht, wk) -> wv -> wq -> (wo later) ----
            ht_s, wk_s, wv_s, wq_s = [], [], [], []
            for i in range(NT):
                h_t = sb.tile([P, M], BF16, tag=f"ht{i}", name=f"ht{i}")
                nc.sync.dma_start(h_t[:], ht_d.ap()[P * i:P * i + P, :])
                ht_s.append(h_t)
                k_t = sb.tile([P, H], BF16, tag=f"wa{i}", name=f"wk{i}")
                nc.scalar.dma_start(k_t[:], wk_d.ap()[P * i:P * i + P, :])
                wk_s.append(k_t)
            for i in range(NT):
                v_t = sb.tile([P, H], BF16, tag=f"wb{i}", name=f"wv{i}")
                (nc.sync if i % 2 else nc.scalar).dma_start(
                    v_t[:], wv_d.ap()[P * i:P * i + P, :])
                wv_s.append(v_t)
            for i in range(NT):
                q_t = sb.tile([P, H], BF16, tag=f"wq{i}", name=f"wq{i}")
                (nc.scalar if i % 2 else nc.sync).dma_start(
                    q_t[:], wq_d.ap()[P * i:P * i + P, :])
                wq_s.append(q_t)

            # ---- phase 1: k, v projections + S accumulation ----
            # One matmul per (m-tile, head pair g): k_pair^T @ v_pair gives a
            # 2x2 head block matrix; the diagonal blocks are S_2g (rows 0-63)
            # and S_2g+1 (rows 64-127) -- exactly the partition halves the o
            # stage needs. Pairs 0-3 accumulate in bank A, 4-7 in bank B.
            def proj_mtile(dst, w_s, tm):
                for co in range(2):
                    p_t = ps.tile([P, FD], F32, tag="big", bufs=7,
                                  name=f"pp{tm}")
                    for ci in range(NT):
                        nc.tensor.matmul(
                            p_t[:],
                            ht_s[ci][:, P * tm:P * tm + P],
                            w_s[ci][:, FD * co:FD * co + FD],
                            start=(ci == 0), stop=(ci == NT - 1),
                        )
                    nc.vector.tensor_copy(dst[:, FD * co:FD * co + FD], p_t[:])

            # all k projections first: they only need ht+wk (4MB) -- wv
            # streams in under the ~28us of k matmuls, so the PE never starves
            k_tiles = []
            for tm in range(NT):
                k_t = sb.tile([P, H], BF16, tag=f"k{tm}", name=f"k{tm}")
                proj_mtile(k_t, wk_s, tm)
                k_tiles.append(k_t)
            s_psA = ps.tile([P, FD], F32, tag="big", bufs=7, name="s_psA")
            s_psB = ps.tile([P, FD], F32, tag="big", bufs=7, name="s_psB")
            # interleaved per-pair accumulation groups share these banks; a
            # start=True would clear has_written bank-wide and drop other
            # pairs' partials -> zero the banks once and accumulate throughout
            nc.vector.memset(s_psA[:], 0.0)
            nc.vector.memset(s_psB[:], 0.0)
            for tm in range(NT):
                k_t = k_tiles[tm]
                v_t = sb.tile([P, H], BF16, tag="vv", bufs=4, name=f"v{tm}")
                # S pairs g<4 only need v cols 0-511: emit them right after
                # the co=0 chunk so they don't wait on the second v cast
                for co in range(2):
                    p_t = ps.tile([P, FD], F32, tag="big", bufs=7,
                                  name=f"pv{tm}")
                    for ci in range(NT):
                        nc.tensor.matmul(
                            p_t[:],
                            ht_s[ci][:, P * tm:P * tm + P],
                            wv_s[ci][:, FD * co:FD * co + FD],
                            start=(ci == 0), stop=(ci == NT - 1),
                        )
                    nc.vector.tensor_copy(v_t[:, FD * co:FD * co + FD], p_t[:])
                    for g in range(4 * co, 4 * co + 4):
                        bank = s_psA if g < 4 else s_psB
                        cc = P * (g % 4)
                        nc.tensor.matmul(
                            bank[:, cc:cc + P],
                            k_t[:, P * g:P * g + P],
                            v_t[:, P * g:P * g + P],
                            start=False, stop=(tm == NT - 1),
                            skip_group_check=True,
                        )
            s_sbA = sb.tile([P, FD], BF16, tag="ssb", bufs=2, name="s_sbA")
            s_sbB = sb.tile([P, FD], BF16, tag="ssb", bufs=2, name="s_sbB")
            nc.vector.tensor_copy(s_sbA[:], s_psA[:])
            nc.vector.tensor_copy(s_sbB[:], s_psB[:])

            # ---- wo loads (reuse wa slots once wk is done) ----
            wo_s = []
            for i in range(NT):
                o_t = sb.tile([P, H], BF16, tag=f"wa{i}", name=f"wo{i}")
                nc.sync.dma_start(o_t[:], wo_d.ap()[P * i:P * i + P, :])
                wo_s.append(o_t)

            # ---- phase 2: qT projection + oT, software-pipelined so the
            # oT matmuls of iteration `to` hide under the q matmuls of
            # iteration `to+1` (the bf16 cast of qT is on the path) ----
            qt_tiles = [None] * NT
            ot_s = []

            def emit_q(to):
                q_t = sb.tile([P, M], BF16, tag="qt", bufs=3, name=f"qt{to}")
                for cm in range(2):
                    p_t = ps.tile([P, FD], F32, tag="big", bufs=7, name=f"pq{to}")
                    for ci in range(NT):
                        nc.tensor.matmul(
                            p_t[:],
                            wq_s[ci][:, P * to:P * to + P],
                            ht_s[ci][:, FD * cm:FD * cm + FD],
                            start=(ci == 0), stop=(ci == NT - 1),
                        )
                    nc.vector.tensor_copy(q_t[:, FD * cm:FD * cm + FD], p_t[:])
                qt_tiles[to] = q_t

            def emit_o(to):
                q_t = qt_tiles[to]
                o_t = sb.tile([P, M], BF16, tag=f"ot{to}", name=f"ot{to}")
                for cm in range(2):
                    p_t = ps.tile([P, FD], F32, tag="big", bufs=7, name=f"po{to}")
                    for pp in range(2):
                        s_bank = s_sbA if to < 4 else s_sbB
                        cc = P * (to % 4) + D * pp
                        nc.tensor.matmul(
                            p_t[D * pp:D * pp + D, :],
                            s_bank[D * pp:D * pp + D, cc:cc + D],
                            q_t[D * pp:D * pp + D, FD * cm:FD * cm + FD],
                            start=True, stop=True,
                        )
                    nc.vector.tensor_copy(o_t[:, FD * cm:FD * cm + FD], p_t[:])
                ot_s.append(o_t)

            for to in range(NT):
                emit_q(to)
                if to > 0:
                    emit_o(to - 1)
            emit_o(NT - 1)

            # ---- phase 3: out = oT.T @ wo ----
            for tm in range(NT):
                o_sb = sb.tile([P, H], F32, tag=f"wb{tm}", name=f"osb{tm}")
                for cj in range(2):
                    p_t = ps.tile([P, FD], F32, tag="big", bufs=7, name=f"pf{tm}")
                    for to in range(NT):
                        nc.tensor.matmul(
                            p_t[:],
                            ot_s[to][:, P * tm:P * tm + P],
                            wo_s[to][:, FD * cj:FD * cj + FD],
                            start=(to == 0), stop=(to == NT - 1),
                        )
                    nc.vector.tensor_copy(o_sb[:, FD * cj:FD * cj + FD], p_t[:])
                    if tm == NT - 1:
                        # last tile: quarter-chunk stores so the end-of-kernel
                        # DMA wait gates on a 128KB transfer, not 256KB
                        for qq in range(2):
                            off = FD * cj + 256 * qq
                            (nc.sync if qq else nc.scalar).dma_start(
                                out_d.ap()[P * tm:P * tm + P, off:off + 256],
                                o_sb[:, off:off + 256])
                    else:
                        (nc.sync if cj else nc.scalar).dma_start(
                            out_d.ap()[P * tm:P * tm + P, FD * cj:FD * cj + FD],
                            o_sb[:, FD * cj:FD * cj + FD])

    nc.compile()
    return nc


def _get_nc():
    if "nc" not in _CACHE:
        _CACHE["nc"] = _build()
    return _CACHE["nc"]


def _run(h, Wq, Wk, Wv, Wo, trace=False):
    nc = _get_nc()
    bf16 = ml_dtypes.bfloat16
    wq = np.ascontiguousarray(np.asarray(Wq).T).astype(bf16)
    wk = np.ascontiguousarray(np.asarray(Wk).T).astype(bf16)
    wv = np.ascontiguousarray(np.asarray(Wv).T).astype(bf16)
    wo = np.ascontiguousarray(np.asarray(Wo).T).astype(bf16)
    in_maps = []
    for b in range(NC):
        in_maps.append({
            "ht": np.ascontiguousarray(np.asarray(h[b]).T).astype(bf16),
            "wq": wq, "wk": wk, "wv": wv, "wo": wo,
        })
    res = run_bass_kernel_spmd(nc, in_maps, core_ids=list(range(NC)), trace=trace)
    out = np.stack([res.results[b]["out"] for b in range(NC)], axis=0)
    return out, res


def kernel(h, key_pe, Wq, Wk, Wv, Wo):
    # key_pe only feeds the reference's dead softmax branch; unused.
    out, _ = _run(h, Wq, Wk, Wv, Wo)
    return out

